# revision 1
# baseline (speedup 1.0000x reference)
"""Trainium2 Bass kernel for nn_Decode (3-step Time-LSTM decoder + dense stack).

Sharding: pure data parallel over batch across 8 NeuronCores (4096 rows each),
weights replicated. Device layout is feature-major (transposed): activations
are [feature_partition, batch_free] tiles, so all weights stay PE-stationary
and batch streams as the matmul moving operand (N=512 columns per chunk = one
PSUM bank at fp32).

Host-side prep (sharding/layout only):
  - slice context_state[:, 2, :] (the model reads only step 2)
  - fold the per-step attention vectors into Wx/Wxt:
        (h*aw_t) @ W == h @ (aw_t[:,None]*W)
  - transpose h to [HID, B] so the device reads feature-major data

All matmuls run as float32r (TF32-like: 1 col/cycle at N>=256; measured
~1.4e-4 rel err/matmul vs 2.2e-3 for bf16). Per step t (banks i,f,o | Tg,g):
  gates = Wk_t.T @ h_last (+ Uh.T @ h_t) (+ Wto.T t into o-bank)
          (+ sigma(Wtt.T t) accumulated into the Tg bank via identity matmul;
           sigma(Wtt_j*t_b) itself is a GpSimd partition_broadcast of t plus
           one ACT sigmoid with a per-partition scale - no PE/PSUM needed)
  c' = f*c + i*Tg*tanh(g);  h' = o*tanh(c');  out_t = relu-dense x3 (h')

Schedule notes (what made it fast - 259us -> 120us on the cost model):
  - STEP-MAJOR loop (for t: for chunk:): consecutive uses of the PSUM gate
    banks belong to different, independent batch chunks, so each chunk's
    h-recurrence latency hides behind the other chunks' gate matmuls.
  - PSUM split [i|f|o] + [Tg|g]: the 3-bank fused sigmoid is not gated by
    the S-chain, and banks recycle in two independent short cycles.
  - ACT is the binding in-order stream (~95us busy of ~120us): sigma(i,f,o)
    is emitted first per instance, the S-chain after it; relus run on DVE;
    p1/p3 products run on GpSimd to unload DVE.
  - t tiles are prefetched two instances ahead on the Pool DMA queue; bulk
    h loads are split per chunk on the sync queue in need-by order.

Fast path requires all-zero biases (true for this problem); a generic path
applies biases through ACT's per-partition bias operand.
"""
import sys

sys.path.insert(0, "/opt/trn_rl_repo")

import numpy as np
import concourse.bacc as bacc
import concourse.tile as tile
from concourse import mybir
from concourse.bass_utils import run_bass_kernel_spmd

N_CORES = 8
B = 32768
HID = 256
FEAT = 128
R = B // N_CORES        # batch rows per core
NB = 512                # batch columns per chunk (= one PSUM bank at fp32)
NCHUNK = R // NB
F32R = mybir.dt.float32r
F32 = mybir.dt.float32
BF16 = mybir.dt.bfloat16
AF = mybir.ActivationFunctionType

DEFAULT_CFG = dict(
    zero_bias=True,   # fused sigmoid across gate banks (requires zero biases)
    relu_act=0,       # of each 6 dense relus, how many run on ACT (rest on DVE)
    gate_dt="f32",    # dtype of gate/elementwise tiles: "f32" | "bf16"
    act_bufs=2,       # default buffering of the activation tile pool
    ifoT_bufs=3,      # buffering of the gate-output tiles
    fine_psum=False,  # gate banks as [i|f],[o],[Tg|g] tiles instead of [i|f|o],[Tg|g]
    pool_tt="13",    # which cell-update products run on GpSimd: subset of "13h"
)


def build_nc(cfg=None):
    cfg = {**DEFAULT_CFG, **(cfg or {})}
    zero_bias = cfg["zero_bias"]
    relu_act = cfg["relu_act"]
    act_bufs = cfg["act_bufs"]
    ifoT_bufs = cfg["ifoT_bufs"]
    fine_psum = cfg["fine_psum"]
    pool_tt = cfg["pool_tt"]
    delay_tail = cfg.get("delay_tail", False)
    merge4b = cfg.get("merge4b", False)
    GDT = F32 if cfg["gate_dt"] == "f32" else BF16

    nc = bacc.Bacc(target_bir_lowering=False)

    h_d = nc.dram_tensor("h", [2, 128, R], F32R, kind="ExternalInput")
    wk_d = nc.dram_tensor("wk", [2, 128, 3, 640], F32R, kind="ExternalInput")
    uh_d = nc.dram_tensor("uh", [128, 4, 128], F32R, kind="ExternalInput")
    dw_d = nc.dram_tensor("dw", [128, 3, 128], F32R, kind="ExternalInput")
    trow_d = nc.dram_tensor("trow", [1, 2, 128], F32R, kind="ExternalInput")
    ident_d = nc.dram_tensor("ident", [128, 128], F32R, kind="ExternalInput")
    bias_d = nc.dram_tensor("bias", [128, 9], F32, kind="ExternalInput")
    t_d = nc.dram_tensor("t", [1, 3, R], F32R, kind="ExternalInput")
    out_d = nc.dram_tensor("out", [3, 128, R], F32R, kind="ExternalOutput")

    with tile.TileContext(nc) as tc:
        with (
            tc.tile_pool(name="const", bufs=1) as const,
            tc.tile_pool(name="act", bufs=act_bufs) as act,
            tc.tile_pool(name="ps", bufs=1, space="PSUM") as ps,
        ):
            # Pool (SWDGE) queue carries only trow + the per-instance t tiles,
            # so the first instance's Wto/S chain is ready almost immediately
            trow_sb = const.tile([1, 2, 128], F32R)
            nc.gpsimd.dma_start(out=trow_sb[:], in_=trow_d[:])
            # warm the ACT table set (sigmoid/tanh/relu) before data arrives
            warm = const.tile([1, 1], F32)
            nc.vector.memset(warm[:], 0.0)
            nc.scalar.activation(warm[:], warm[:], AF.Sigmoid)
            wk_sb = const.tile([128, 2, 3, 640], F32R)
            hsb = const.tile([128, 2, R], F32R)
            ident_sb = const.tile([128, 128], F32R)
            bias_sb = const.tile([128, 9], F32)
            dw_sb = const.tile([128, 3, 128], F32R)
            uh_sb = const.tile([128, 4, 128], F32R)
            wk_r = wk_d.rearrange("a p t m -> p a t m")
            h_r = h_d.rearrange("a p n -> p a n")
            # sync queue in need-by order: wk[t0] (finely split) + h0 first,
            # then ident/dense weights, remaining h chunks, later-step weights
            for m in range(5):
                nc.sync.dma_start(out=wk_sb[:, :, 0, m * 128:(m + 1) * 128],
                                  in_=wk_r[:, :, 0, m * 128:(m + 1) * 128])
                if m == 0:
                    nc.sync.dma_start(out=hsb[:, :, 0:NB], in_=h_r[:, :, 0:NB])
            nc.sync.dma_start(out=ident_sb[:], in_=ident_d[:])
            nc.sync.dma_start(out=dw_sb[:], in_=dw_d[:])
            for c in range(1, NCHUNK):
                col = slice(c * NB, (c + 1) * NB)
                nc.sync.dma_start(out=hsb[:, :, col], in_=h_r[:, :, col])
                if c == 1:
                    nc.sync.dma_start(out=uh_sb[:], in_=uh_d[:])
                    nc.sync.dma_start(out=bias_sb[:], in_=bias_d[:])
                if c == 2:
                    nc.sync.dma_start(out=wk_sb[:, :, 1, :], in_=wk_r[:, :, 1, :])
                if c == 4:
                    nc.sync.dma_start(out=wk_sb[:, :, 2, :], in_=wk_r[:, :, 2, :])

            # recurrent state, updated in place (the write of step t happens
            # after all step-t readers of the same column range)
            h_st = const.tile([128, R], F32R, name="hst")
            c_st = const.tile([128, R], GDT, name="cst")

            t_tiles = {}

            def load_t(t, inst):
                col, nb, key = inst
                tt = act.tile([1, nb], F32R, tag="t_tile", bufs=4,
                              name=f"tt_{key}_{t}")
                nc.gpsimd.dma_start(out=tt[:], in_=t_d[:, t, col])
                t_tiles[(t, key)] = tt

            def emit_gates(t, inst, h_prev):
                """PE gate matmuls for one chunk -> (gsig, tgg, t_tile)."""
                col, nb, key = inst
                c = key
                t_tile = t_tiles.pop((t, key))
                if merge4b:
                    gsig = ps.tile([128, 4, nb], F32, tag="gsig", name=f"gsig_{c}_{t}")
                    gg = ps.tile([128, nb], F32, tag="gg", name=f"gg_{c}_{t}")
                    tgg = (gsig, gg)
                    targets = [gsig[:, 0, :], gsig[:, 1, :], gsig[:, 2, :],
                               gsig[:, 3, :], gg[:]]
                elif fine_psum:
                    if_ps = ps.tile([128, 2, nb], F32, tag="ifp", name=f"ifp_{c}_{t}")
                    o_ps = ps.tile([128, nb], F32, tag="op", name=f"op_{c}_{t}")
                    tgg = ps.tile([128, 2, nb], F32, tag="tgg", name=f"tgg_{c}_{t}")
                    gsig = (if_ps, o_ps)
                    targets = [if_ps[:, 0, :], if_ps[:, 1, :], o_ps[:],
                               tgg[:, 0, :], tgg[:, 1, :]]
                else:
                    gsig = ps.tile([128, 3, nb], F32, tag="gsig", name=f"gsig_{c}_{t}")
                    tgg = ps.tile([128, 2, nb], F32, tag="tgg", name=f"tgg_{c}_{t}")
                    targets = [gsig[:, 0, :], gsig[:, 1, :], gsig[:, 2, :],
                               tgg[:, 0, :], tgg[:, 1, :]]
                for m in range(5):
                    tgt = targets[m]
                    n_extra = (1 if m == 2 else 0) \
                        + (1 if (t > 0 and m != 3) else 0) \
                        + (1 if m == 3 else 0)
                    for k in range(2):
                        nc.tensor.matmul(
                            tgt,
                            wk_sb[:, k, t, m * 128:(m + 1) * 128],
                            hsb[:, k, col],
                            start=(k == 0),
                            stop=(k == 1 and n_extra == 0),
                        )
                    if m == 2:  # o += Wto.T t
                        n_extra -= 1
                        nc.tensor.matmul(
                            tgt, trow_sb[:, 1, :], t_tile[:],
                            start=False, stop=(n_extra == 0),
                        )
                    if t > 0 and m != 3:  # += Uh.T h_prev
                        n_extra -= 1
                        nc.tensor.matmul(
                            tgt, uh_sb[:, min(m, 3), :], h_prev[:, col],
                            start=False, stop=(n_extra == 0),
                        )
                return gsig, tgg, t_tile

            def emit_schain(t, c, nb, tgg, t_tile):
                """sigma(Wtt.T t) via GpSimd broadcast + per-partition ACT scale,
                accumulated into the Tg bank with an identity matmul."""
                tb = act.tile([128, nb], F32R, tag="tb", bufs=2, name=f"tb_{c}_{t}")
                nc.gpsimd.partition_broadcast(tb[:], t_tile[:])
                s_sb = act.tile([128, nb], F32R, tag="s_sb", bufs=2, name=f"s_{c}_{t}")
                nc.scalar.activation(s_sb[:], tb[:], AF.Sigmoid,
                                     scale=bias_sb[:, 8:9])
                tg_bank = tgg[0][:, 3, :] if merge4b else tgg[:, 0, :]
                nc.tensor.matmul(tg_bank, ident_sb[:], s_sb[:],
                                 start=False, stop=True)

            def emit_sigmas(t, c, nb, gsig, tgg, t_tile, ifoT_dst, g_dst):
                """PSUM gate banks -> sigmoid/tanh -> SBUF slices."""
                if merge4b and zero_bias:
                    gsig4, gg = tgg
                    emit_schain(t, c, nb, tgg, t_tile)
                    nc.scalar.activation(ifoT_dst[:], gsig4[:], AF.Sigmoid)
                    nc.scalar.activation(g_dst, gg[:], AF.Tanh)
                elif fine_psum and zero_bias:
                    if_ps, o_ps = gsig
                    nc.scalar.activation(ifoT_dst[:, 0:2, :], if_ps[:], AF.Sigmoid)
                    nc.scalar.activation(ifoT_dst[:, 2, :], o_ps[:], AF.Sigmoid)
                    emit_schain(t, c, nb, tgg, t_tile)
                    nc.scalar.activation(ifoT_dst[:, 3, :], tgg[:, 0, :], AF.Sigmoid)
                    nc.scalar.activation(g_dst, tgg[:, 1, :], AF.Tanh)
                elif zero_bias:
                    nc.scalar.activation(ifoT_dst[:, 0:3, :], gsig[:], AF.Sigmoid)
                    emit_schain(t, c, nb, tgg, t_tile)
                    nc.scalar.activation(ifoT_dst[:, 3, :], tgg[:, 0, :], AF.Sigmoid)
                    nc.scalar.activation(g_dst, tgg[:, 1, :], AF.Tanh)
                else:
                    if fine_psum:
                        if_ps, o_ps = gsig
                        srcs = [if_ps[:, 0, :], if_ps[:, 1, :], o_ps[:], tgg[:, 0, :]]
                    else:
                        srcs = [gsig[:, 0, :], gsig[:, 1, :], gsig[:, 2, :], tgg[:, 0, :]]
                    emit_schain(t, c, nb, tgg, t_tile)
                    for m in range(4):
                        nc.scalar.activation(ifoT_dst[:, m, :], srcs[m], AF.Sigmoid,
                                             bias=bias_sb[:, m:m + 1])
                    nc.scalar.activation(g_dst, tgg[:, 1, :], AF.Tanh,
                                         bias=bias_sb[:, 4:5])

            def emit_dense(t, inst, h_cur):
                col, nb, c = inst
                ci = int(c.rstrip("ab"))
                cur = None
                for l in range(3):
                    dps = ps.tile([128, nb], F32, tag="dps", bufs=3,
                                  name=f"dps_{c}_{t}_{l}")
                    nc.tensor.matmul(
                        dps[:], dw_sb[:, l, :],
                        h_cur[:, col] if l == 0 else cur[:],
                        start=True, stop=True,
                    )
                    dsb = act.tile([128, nb], F32R, tag=f"dsb{l}", bufs=3, name=f"d_{c}_{t}_{l}")
                    if not zero_bias:
                        nc.scalar.activation(
                            dsb[:], dps[:], AF.Relu, bias=bias_sb[:, 5 + l:6 + l]
                        )
                    elif (ci * 3 + t * 5 + l) % 6 < relu_act:
                        nc.scalar.activation(dsb[:], dps[:], AF.Relu)
                    else:
                        nc.vector.tensor_relu(dsb[:], dps[:])
                    cur = dsb
                nc.sync.dma_start(out=out_d[t, :, col], in_=cur[:])

            # t tiles are prefetched two instances ahead (inside the tail) so
            # the Pool queue never blocks the current instance's S chain.
            # The very last instance is split in half to shorten the serial
            # drain at the end of the kernel.
            def make_insts(t):
                full = [(slice(c * NB, (c + 1) * NB), NB, f"{c}") for c in range(NCHUNK)]
                if t == 2 and cfg.get("split_last", False):
                    c = NCHUNK - 1
                    h0 = slice(c * NB, c * NB + NB // 2)
                    h1 = slice(c * NB + NB // 2, (c + 1) * NB)
                    return full[:-1] + [(h0, NB // 2, f"{c}a"), (h1, NB // 2, f"{c}b")]
                return full

            inst_order = [(tt_, inst) for tt_ in range(3) for inst in make_insts(tt_)]
            load_t(*inst_order[0])
            load_t(*inst_order[1])
            gflat = [0]

            for t in range(3):
                h_prev = h_cur = h_st
                c_prev = c_cur = c_st
                for inst in make_insts(t):
                    col, nb, key = inst
                    gsig, tgg, t_tile = emit_gates(t, inst, h_prev)
                    ifoT = act.tile([128, 4, nb], GDT, tag="ifoT", bufs=ifoT_bufs,
                                    name=f"ifoT_{key}_{t}")
                    g2 = act.tile([128, nb], GDT, tag="g2", name=f"g2_{key}_{t}")
                    emit_sigmas(t, key, nb, gsig, tgg, t_tile, ifoT[:], g2[:])

                    # ---- DVE: cell update ----
                    p1 = act.tile([128, nb], GDT, tag="p1", name=f"p1_{key}_{t}")
                    eng1 = nc.gpsimd if "1" in pool_tt else nc.vector
                    eng1.tensor_mul(p1[:], ifoT[:, 3, :], g2[:])
                    if t == 0:
                        nc.vector.tensor_mul(c_cur[:, col], ifoT[:, 0, :], p1[:])
                    else:
                        p2 = act.tile([128, nb], GDT, tag="p2", name=f"p2_{key}_{t}")
                        nc.vector.tensor_mul(p2[:], ifoT[:, 0, :], p1[:])
                        p3 = act.tile([128, nb], GDT, tag="p3", name=f"p3_{key}_{t}")
                        eng3 = nc.gpsimd if "3" in pool_tt else nc.vector
                        eng3.tensor_mul(p3[:], ifoT[:, 1, :], c_prev[:, col])
                        nc.vector.tensor_add(c_cur[:, col], p2[:], p3[:])
                    tanh_c = act.tile([128, nb], GDT, tag="tanh_c", name=f"tc_{key}_{t}")
                    nc.scalar.activation(tanh_c[:], c_cur[:, col], AF.Tanh)
                    engh = nc.gpsimd if "h" in pool_tt else nc.vector
                    engh.tensor_mul(h_cur[:, col], ifoT[:, 2, :], tanh_c[:])

                    gflat_now = gflat[0]
                    gflat[0] += 1
                    if gflat_now + 2 < len(inst_order):
                        load_t(*inst_order[gflat_now + 2])
                    emit_dense(t, inst, h_cur)

    nc.finalize()
    return nc


_NC_CACHE = {}


def _get_nc(key, cfg):
    if key not in _NC_CACHE:
        _NC_CACHE[key] = build_nc(cfg)
    return _NC_CACHE[key]


def kernel(context_state, input_t, aw1, aw2, aw3, Wx, Uh, b,
           Wxt, Wtt, bt, Wto, w1, b1, w2, b2, w3, b3):
    f32 = np.float32
    f64 = np.float64

    # ---- host-side prep / sharding ----
    h_last = np.asarray(context_state)[:, 2, :].astype(f32)          # [B, HID]
    hT = np.ascontiguousarray(h_last.T).reshape(2, 128, B)           # [2,128,B]
    tT = np.ascontiguousarray(np.asarray(input_t)[:, 3:, 0].T)       # [3, B]
    aw = np.concatenate(
        [np.asarray(aw1), np.asarray(aw2), np.asarray(aw3)], axis=1
    )[0].astype(f64)                                                 # [3, HID]

    Wx64, Wxt64 = np.asarray(Wx, f64), np.asarray(Wxt, f64)
    wk = np.empty((HID, 3, 640), f64)
    for t in range(3):
        wxf = aw[t][:, None] * Wx64                                  # [HID, 512]
        wtf = aw[t][:, None] * Wxt64                                 # [HID, 128]
        wk[:, t, 0:384] = wxf[:, 0:384]      # i, f, o
        wk[:, t, 384:512] = wtf              # Tg
        wk[:, t, 512:640] = wxf[:, 384:512]  # g
    wk = np.ascontiguousarray(wk.astype(f32)).reshape(2, 128, 3, 640)

    uh = np.ascontiguousarray(np.asarray(Uh, f32).reshape(128, 4, 128))
    dw = np.ascontiguousarray(np.stack(
        [np.asarray(w1, f32), np.asarray(w2, f32), np.asarray(w3, f32)], axis=1))
    trow = np.ascontiguousarray(
        np.stack([np.asarray(Wtt, f32)[0], np.asarray(Wto, f32)[0]], axis=0)
    ).reshape(1, 2, 128)
    ident = np.eye(128, dtype=f32)
    bias = np.ascontiguousarray(np.stack(
        [np.asarray(b, f32)[0:128], np.asarray(b, f32)[128:256],
         np.asarray(b, f32)[256:384], np.asarray(bt, f32),
         np.asarray(b, f32)[384:512], np.asarray(b1, f32),
         np.asarray(b2, f32), np.asarray(b3, f32),
         np.asarray(Wtt, f32)[0]], axis=1))                          # [128, 9]

    zero_bias = not (bias[:, 0:8].any())
    cfg = dict(DEFAULT_CFG, zero_bias=zero_bias)
    nc = _get_nc(("main", zero_bias), cfg)

    in_maps = []
    for core in range(N_CORES):
        rs = slice(core * R, (core + 1) * R)
        in_maps.append(dict(
            h=np.ascontiguousarray(hT[:, :, rs]),
            wk=wk, uh=uh, dw=dw, trow=trow, ident=ident, bias=bias,
            t=np.ascontiguousarray(tT[:, rs]).reshape(1, 3, R),
        ))

    global _LAST_IN_MAPS
    _LAST_IN_MAPS = in_maps
    res = run_bass_kernel_spmd(nc, in_maps, core_ids=list(range(N_CORES)))
    outs = [np.transpose(res.results[c]["out"], (2, 0, 1)) for c in range(N_CORES)]
    return np.ascontiguousarray(np.concatenate(outs, axis=0))

